# revision 44
# baseline (speedup 1.0000x reference)
"""Trainium2 Bass kernel for an AttentionBlock (GroupNorm + single-head
self-attention + residual) over x[8, 512, 64, 64].

Sharding: data-parallel over batch - one batch element per NeuronCore (8
cores), no collectives.  Per-core layout is channel-major [C=512, N=4096].

All heavy matmuls run as fp8(TRN e4m3) DoubleRow (perf-mode) matmuls:
effective K=256 per instruction at 2 moving rows/cycle - measured ~2.5x
the f32r MAC rate on this silicon (~137ns compute + ~80ns weight load per
512-row instruction; the 512-row output cap is an ISA limit, so ~215ns
per matmul is the per-instruction floor).

Structure (vs the f32r baseline this replaced):
  - GroupNorm is folded into the *activations*: one DVE pass makes
    xn = a*x + beff in fp8 pair layout [p, ct2, i, m].  All weights are
    then constants, pre-quantized to fp8 pair layout on the host.
  - The output projection is folded into V on the host (wvo = wvT @ woT),
    so V'' = xn @ wvo directly; no second projection on device.  bo/bv
    fold into a final additive constant (softmax weights sum to 1).
  - The K-side bias bk cancels in the softmax; only bq survives (on Q').
  - x is loaded once as bf16 (halves the DMA serial head; stats/xn/
    residual all read the resident copy).  The xn pass is split ACT/DVE
    (2 channel tiles each) so DVE doesn't pace phase 3.
  - exp evicts fp8 with a constant offset exp(s*scale - 2.5): cancels
    exactly in the softmax, keeps max P ~ e^3.6 << 240 (TRN e4m3
    overflows to Inf at 256).
  - Softmax denominator: 12 of 16 key pair-chunks accumulate on DVE
    (fp8 P pairs, one 1024-wide add each); 4 accumulate on the PE via a
    broadcast ones-matmul into a held PSUM bank.  A single f32r
    ones-matmul folds the DVE partials in (partition-reduce + broadcast),
    and the reciprocal is exp(-ln(dn)) on ACT (DVE reciprocal is ~6.6
    cycles/elem; DVE divide / custom-DVE ISA don't pass this walrus).
    The last block runs its denominator fully on the PE so its exposed
    tail skips the DVE fold chain.
  - PSUM: 4 scores banks (cycling, shared with Q-projection and the
    f32r fold) + 1 held denominator bank + 4 PV accumulators taken
    across the whole query block.
  - y = O*rb + boeff2 + x on DVE straight from PSUM, DMA'd per channel
    tile across the three DMA queues (SP/ACT/gpsimd).

Engine budget per core (of ~347us): PE ~287us (the bottleneck; >95%
busy steady-state at the chip's ~86% power-throttle duty), DVE ~230us,
ACT ~202us.  Serial head ~35us (x DMA on 3 queues + GroupNorm stats),
drain tail ~18us.

Measured (8 cores, NTFF): ~346us HW exec, rel err 5.1e-3 vs fp32
reference (gate 2e-2; error budget is dominated by fp8 rounding of the
attention path, attenuated ~40x by the residual).  f32r baseline was
776-918us.
"""

import os

import ml_dtypes
import numpy as np

import concourse.bass as bass
import concourse.mybir as mybir
import concourse.tile as tile

from concourse.bass_utils import run_bass_kernel_spmd
from concourse.vector_clock import ScopedClock

AF = mybir.ActivationFunctionType
ALU = mybir.AluOpType
FP32 = mybir.dt.float32
F32R = mybir.dt.float32r
FP8 = mybir.dt.float8e4
BF16 = mybir.dt.bfloat16
PM = mybir.MatmulPerfMode.DoubleRow

B = 8
C = 512
N = 4096          # H*W
G = 8             # groups
EPS = 1e-5
CT = C // 128     # 4 channel tiles
NBS = 512         # query-block size
NB = N // NBS     # 8 query blocks
MC2 = N // 256    # 16 key pair-chunks
SCALE = 1.0 / np.sqrt(np.float32(C))
EXP_OFF = 2.5     # exp(s*SCALE - EXP_OFF); cancels in softmax exactly

DEBUG_DUMP = os.environ.get("ATTN_DEBUG_DUMP", "0") == "1"


class _TileContext(tile.TileContext):
    """This container's walrus rejects >1 sync wait on a CTRL instruction
    ("Too many sync wait commands"); split the tail drain's waits across
    multiple drain instructions.  It also rejects long semaphore-range-clear
    ISA instructions ("ISA wrong length"); clear in chunks of <=3."""

    def _drain_and_barrier(self, tick_clock, wait_clock):
        drain_inst = self.nc.sync.drain()
        wait_clock.add_sem_waits(
            drain_inst.ins, ScopedClock({None: tick_clock.global_clock})
        )
        si = drain_inst.ins.sync_info
        if si is not None and si.on_wait and len(si.on_wait) > 1:
            waits = list(si.on_wait)
            drain_inst.ins.sync_info = mybir.SyncInfo(
                on_wait=[waits[0]], on_update=list(si.on_update)
            )
            for w in waits[1:]:
                d = self.nc.sync.drain()
                d.ins.sync_info = mybir.SyncInfo(on_wait=[w], on_update=[])

        self.nc.all_engine_barrier()
        assert self.sems is not None
        popped = self.nc._tile_sem_poison_stack.pop()
        assert popped is self._sem_poison
        sems = list(self.sems.allocated().values())
        for i in range(0, len(sems), 3):
            self.nc.clear_and_free_semaphores(sems[i:i + 3])
        self.nc.all_engine_barrier()


def _split_multi_waits(nc, limit=1):
    """This container's walrus accepts at most one sync wait per instruction.
    Hoist extra waits onto same-engine EventSemaphore instructions inserted
    just before - equivalent ordering (engines execute in program order)."""
    nid = 0
    for f in nc.m.functions:
        for bb in f.blocks:
            out = []
            changed = False
            for inst in bb.instructions:
                si = inst.sync_info
                if si is not None and si.on_wait and len(si.on_wait) > limit:
                    waits = list(si.on_wait)
                    for w in waits[:-limit]:
                        ev = mybir.InstEventSemaphore(
                            name=f"I-wsplit-{nid}",
                            engine=inst.engine,
                            sync_info=mybir.SyncInfo(on_wait=[w], on_update=[]),
                        )
                        nid += 1
                        out.append(ev)
                    inst.sync_info = mybir.SyncInfo(
                        on_wait=waits[-limit:], on_update=list(si.on_update)
                    )
                    changed = True
                out.append(inst)
            if changed:
                bb.instructions = out


def _build_kernel():
    nc = bass.Bass()

    x = nc.declare_dram_parameter("x", [C, N], BF16, isOutput=False)
    # fp8 pair-layout weights: [p, ct2, i, d], channel c = ct2*256 + i*128 + p
    wq8 = nc.declare_dram_parameter("wq8", [128, 2, 2, C], FP8, isOutput=False)
    wk8 = nc.declare_dram_parameter("wk8", [128, 2, 2, C], FP8, isOutput=False)
    wvo8 = nc.declare_dram_parameter("wvo8", [128, 2, 2, C], FP8, isOutput=False)
    gnw = nc.declare_dram_parameter("gnw", [128, CT], FP32, isOutput=False)
    gnb = nc.declare_dram_parameter("gnb", [128, CT], FP32, isOutput=False)
    bqp = nc.declare_dram_parameter("bqp", [128, CT], FP32, isOutput=False)
    bo2 = nc.declare_dram_parameter("bo2", [128, CT], FP32, isOutput=False)
    # group-indicator constants for the cross-partition GroupNorm reductions
    ind128 = nc.declare_dram_parameter("ind128", [128, 2], FP32, isOutput=False)
    indT2 = nc.declare_dram_parameter("indT2", [128, 128], FP32, isOutput=False)
    y = nc.declare_dram_parameter("y", [C, N], FP32, isOutput=True)
    dbg = {}
    if DEBUG_DUMP:
        for nm, shp in [
            ("dbg_ab", [128, 2 * CT]),     # a_pc | beff
            ("dbg_xn", [128, 512]),        # xn8[:, 0, 0, :512]
            ("dbg_k", [128, 512]),         # k8[:, 0, 0, :512]
            ("dbg_q", [128, 512]),         # q8 block0 [:, 0, 0, :]
            ("dbg_v", [128, 512]),         # v2[:, 0, 0, :]
            ("dbg_p", [128, 1024]),        # pb8 block0 mc2=0
            ("dbg_dn", [128, 512]),        # dn2 folded, block 0
            ("dbg_rb", [128, 512]),        # reciprocal broadcast, block 0
        ]:
            dbg[nm] = nc.declare_dram_parameter(nm, shp, FP32, isOutput=True)

    x_r = x[:].rearrange("(t p) m -> t p m", p=128)   # [4, 128, 4096]
    y_r = y[:].rearrange("(t p) m -> t p m", p=128)

    with _TileContext(nc) as tc:
        with (
            tc.tile_pool(name="small", bufs=1) as small,
            tc.tile_pool(name="big", bufs=1) as big,
        ):
            # ---------------- persistent tiles ----------------
            x_sb = big.tile([128, CT, N], BF16, tag="x")       # 32KB/part
            xn8 = big.tile([128, 2, 2, N], FP8, tag="xn")      # 16KB/part
            k8 = big.tile([128, 2, 2, N], FP8, tag="k8")       # 16KB/part
            v2 = big.tile([128, MC2, 2, C], FP8, tag="v2")     # 16KB/part
            wq_sb = small.tile([128, 2, 2, C], FP8, tag="wq8")
            wk_sb = small.tile([128, 2, 2, C], FP8, tag="wk8")
            wvo_sb = small.tile([128, 2, 2, C], FP8, tag="wvo8")

            # x loads: 16 chunks of [128, 1024] spread over 4 DMA queues so
            # the stats head is DMA-limited as briefly as possible.
            qs_eng = [nc.sync, nc.scalar, nc.gpsimd]
            for ct in range(CT):
                for h in range(2):
                    hs = slice(h * 2048, (h + 1) * 2048)
                    qs_eng[(ct * 2 + h) % 3].dma_start(
                        out=x_sb[:, ct, hs], in_=x_r[ct][:, hs]
                    )

            nc.sync.dma_start(out=wq_sb, in_=wq8[:])
            nc.sync.dma_start(out=wk_sb, in_=wk8[:])
            nc.sync.dma_start(out=wvo_sb, in_=wvo8[:])

            ind128_sb = small.tile([128, 2], FP32, tag="ind128")
            indT2_sb = small.tile([128, 128], FP32, tag="indT2")
            nc.gpsimd.dma_start(out=ind128_sb, in_=ind128[:])
            nc.gpsimd.dma_start(out=indT2_sb, in_=indT2[:])

            def load_pc(name, dram):
                t = small.tile([128, CT], FP32, tag=name)
                nc.gpsimd.dma_start(out=t, in_=dram[:])
                return t

            gnw_sb = load_pc("gnw", gnw)
            gnb_sb = load_pc("gnb", gnb)
            bq_sb = load_pc("bqp", bqp)
            bo2_sb = load_pc("bo2", bo2)

            eps_sb = small.tile([128, 1], FP32, tag="eps")
            nc.vector.memset(eps_sb, EPS)
            cbias = small.tile([128, 1], FP32, tag="cbias")
            nc.vector.memset(cbias, -float(EXP_OFF))
            ones_f = small.tile([128, 128], FP32, tag="ones_f")
            nc.vector.memset(ones_f, 1.0)
            ones_r = small.tile([128, 128], F32R, tag="ones_r")
            nc.vector.tensor_copy(ones_r, ones_f)
            # fp8 pair-ones for the PE-side denominator accumulation; M=128
            # so the denominator lands broadcast across all 128 partitions
            # (output rows are what matmuls cost, M is free).
            ones8 = small.tile([128, 2, 128], FP8, tag="ones8")
            nc.vector.tensor_copy(ones8[:, 0, :], ones_f)
            nc.vector.tensor_copy(ones8[:, 1, :], ones_f)

            pcs = small.tile([128, 8], FP32, tag="pcs")        # (s,t): s*4+t
            stats128 = small.tile([128, 8], FP32, tag="st128")  # (j,t): j*4+t
            a_pc = small.tile([128, CT], FP32, tag="a_pc")
            beff = small.tile([128, CT], FP32, tag="beff")

            # ============ phase 1: GroupNorm statistics =============
            with (
                tc.tile_pool(name="ps3", bufs=4, space="PSUM") as ps_mm,
                tc.tile_pool(name="sttmp", bufs=4) as sttmp,
            ):
                for ct in range(CT):
                    st = sttmp.tile([128, 8, 6], FP32, tag="st")
                    for j in range(8):
                        nc.vector.bn_stats(
                            out=st[:, j], in_=x_sb[:, ct, j * 512:(j + 1) * 512]
                        )
                    mv = sttmp.tile([128, 2], FP32, tag="mv")
                    nc.vector.bn_aggr(out=mv, in_=st)
                    # pcs[:, ct]=mean ; pcs[:, 4+ct]=E[x^2]=var+mean^2
                    nc.vector.tensor_copy(pcs[:, ct:ct + 1], mv[:, 0:1])
                    m2 = sttmp.tile([128, 1], FP32, tag="m2")
                    nc.vector.tensor_mul(m2, mv[:, 0:1], mv[:, 0:1])
                    nc.vector.tensor_add(
                        pcs[:, 4 + ct:5 + ct], mv[:, 1:2], m2
                    )

                # group sums over the 64 member channels' stats
                gs_ps = ps_mm.tile([128, 512], FP32, tag="mm")
                nc.tensor.matmul(
                    gs_ps[:2, :8], lhsT=ind128_sb, rhs=pcs, start=True, stop=True
                )
                # ind128 carries the 1/64 group-mean scale (host-folded)
                gs_sb = small.tile([128, 8], FP32, tag="gs")
                nc.vector.tensor_copy(gs_sb[:2], gs_ps[:2, :8])
                nc.vector.memset(stats128, 0.0)
                nc.vector.tensor_copy(stats128[:2, 0:4], gs_sb[:2, 0:4])
                vtmp = small.tile([128, 4], FP32, tag="vtmp")
                nc.vector.tensor_mul(vtmp[:2], gs_sb[:2, 0:4], gs_sb[:2, 0:4])
                nc.vector.tensor_sub(
                    stats128[:2, 4:8], gs_sb[:2, 4:8], vtmp[:2]
                )
                nc.scalar.activation(
                    stats128[:2, 4:8], stats128[:2, 4:8], AF.Sqrt,
                    bias=eps_sb[:2],
                )
                nc.vector.reciprocal(stats128[:2, 4:8], stats128[:2, 4:8])

                # broadcast group stats back to channels
                bc_ps = ps_mm.tile([128, 512], FP32, tag="mm")
                nc.tensor.matmul(
                    bc_ps[:, :8], lhsT=indT2_sb, rhs=stats128,
                    start=True, stop=True,
                )
                # a = rstd * gn_w ; beff = gn_b - mean * a  (PSUM-direct)
                nc.vector.tensor_mul(a_pc, bc_ps[:, 4:8], gnw_sb)
                nc.vector.tensor_mul(beff, bc_ps[:, 0:4], a_pc)
                nc.vector.tensor_sub(beff, gnb_sb, beff)

            if DEBUG_DUMP:
                ab = small.tile([128, 2 * CT], FP32, tag="dbg_ab")
                nc.vector.tensor_copy(ab[:, 0:CT], a_pc)
                nc.vector.tensor_copy(ab[:, CT:2 * CT], beff)
                nc.sync.dma_start(out=dbg["dbg_ab"][:], in_=ab)

            # ===== phases 2+3 interleaved per 1024-token slab ==========
            ps3b_ctx = tc.tile_pool(name="ps3b", bufs=4, space="PSUM")
            ps_mm = ps3b_ctx.__enter__()
            # xn8[p, ct2, i, m] = a[c]x[c, m] + beff[c],  c = ct2*256+i*128+p
            # then K' / V'' for that slab, so the PE starts ~5us after the
            # GroupNorm statistics instead of after the whole xn pass.
            # K'[d, m] (d = dt*128+p, pair layout over dt) - no bias (bk
            # cancels in softmax).  V''[m, e] = xn^T wvo (pair over tokens).
            for h in range(4):
                subs = ([slice(0, 512), slice(512, 1024)] if h == 0
                        else [slice(0, 1024)])
                for sub in subs:
                    hs = slice(h * 1024 + sub.start, h * 1024 + sub.stop)
                    for ct in range(CT):
                        ct2, i = divmod(ct, 2)
                        if ct < 2:
                            # ACT carries half the xn pass (Identity with
                            # per-partition scale/bias APs): DVE otherwise
                            # paces phase 3 with xn + V'' evictions
                            nc.scalar.activation(
                                xn8[:, ct2, i, hs], x_sb[:, ct, hs],
                                AF.Identity, bias=beff[:, ct:ct + 1],
                                scale=a_pc[:, ct:ct + 1],
                            )
                        else:
                            nc.vector.tensor_scalar(
                                xn8[:, ct2, i, hs], x_sb[:, ct, hs],
                                a_pc[:, ct:ct + 1], beff[:, ct:ct + 1],
                                op0=ALU.mult, op1=ALU.add,
                            )
                for m2 in (2 * h, 2 * h + 1):
                    sl = slice(m2 * 512, (m2 + 1) * 512)
                    for dt in range(CT):
                        kp = ps_mm.tile([128, 512], FP32, tag="mm")
                        for ct2 in range(2):
                            nc.tensor.matmul(
                                kp,
                                lhsT=wk_sb[:, ct2, :, dt * 128:(dt + 1) * 128],
                                rhs=xn8[:, ct2, :, sl],
                                start=(ct2 == 0), stop=(ct2 == 1),
                                perf_mode=PM,
                            )
                        nc.scalar.copy(k8[:, dt // 2, dt % 2, sl], kp)
                    for mt in range(4):
                        mg = m2 * 4 + mt
                        msl = slice(mg * 128, (mg + 1) * 128)
                        vp = ps_mm.tile([128, 512], FP32, tag="mm")
                        for ct2 in range(2):
                            nc.tensor.matmul(
                                vp,
                                lhsT=xn8[:, ct2, :, msl],
                                rhs=wvo_sb[:, ct2],
                                start=(ct2 == 0), stop=(ct2 == 1),
                                perf_mode=PM,
                            )
                        nc.vector.tensor_copy(v2[:, mg // 2, mg % 2, :], vp)

            ps3b_ctx.__exit__(None, None, None)

            if DEBUG_DUMP:
                t = small.tile([128, 512], FP32, tag="dbg_xn")
                nc.vector.tensor_copy(t, xn8[:, 0, 0, 0:512])
                nc.sync.dma_start(out=dbg["dbg_xn"][:], in_=t)

            if DEBUG_DUMP:
                t = small.tile([128, 512], FP32, tag="dbg_k")
                nc.vector.tensor_copy(t, k8[:, 0, 0, 0:512])
                nc.sync.dma_start(out=dbg["dbg_k"][:], in_=t)
                t = small.tile([128, 512], FP32, tag="dbg_v")
                nc.vector.tensor_copy(t, v2[:, 0, 0, :])
                nc.sync.dma_start(out=dbg["dbg_v"][:], in_=t)

            # ============ phase 4: attention per query block ===========
            with (
                tc.tile_pool(name="qp", bufs=3) as qpool,
                tc.tile_pool(name="pp", bufs=8) as ppool,
                tc.tile_pool(name="dn", bufs=3) as dnpool,
                tc.tile_pool(name="yp", bufs=3) as ypool,
                tc.tile_pool(name="ps_S", bufs=3, space="PSUM") as ps_s,
                tc.tile_pool(name="ps_dn", bufs=1, space="PSUM") as ps_dn,
                tc.tile_pool(name="ps_O", bufs=4, space="PSUM") as ps_o,
            ):
                def emit_qproj(nb):
                    """Q' projection for block nb (fp8 pair layout), with
                    +bq folded in on the DVE eviction."""
                    nsl_q = slice(nb * NBS, (nb + 1) * NBS)
                    q8 = qpool.tile([128, 2, 2, NBS], FP8, tag="q",
                                    name=f"q{nb}")
                    for dt in range(CT):
                        qp_ps = ps_s.tile([128, 512], FP32, tag="s",
                                          name=f"qps{nb}_{dt}")
                        for ct2 in range(2):
                            nc.tensor.matmul(
                                qp_ps,
                                lhsT=wq_sb[:, ct2, :, dt * 128:(dt + 1) * 128],
                                rhs=xn8[:, ct2, :, nsl_q],
                                start=(ct2 == 0), stop=(ct2 == 1),
                                perf_mode=PM,
                            )
                        nc.vector.tensor_scalar_add(
                            q8[:, dt // 2, dt % 2, :], qp_ps,
                            bq_sb[:, dt:dt + 1],
                        )
                    return q8

                q8_cur = emit_qproj(0)

                for nb in range(NB):
                    nsl = slice(nb * NBS, (nb + 1) * NBS)
                    q8 = q8_cur

                    o_ps = [
                        ps_o.tile([128, 512], FP32, tag="o", name=f"o_ps{dt}")
                        for dt in range(CT)
                    ]
                    # denominator: even key pair-chunks accumulate on DVE
                    # (dn2), odd ones on the PE via a broadcast ones-matmul
                    # (dn_ps, one held ps_s buffer); combined at the tail by
                    # a f32r ones-matmul that also reduces dn2's partitions.
                    dn_ps = ps_dn.tile([128, 512], FP32, tag="dn",
                                       name=f"dn_ps{nb}")
                    last = nb == NB - 1
                    dn2 = None
                    if not last:
                        dn2 = dnpool.tile([128, 2, NBS], FP32, tag="dn2",
                                          name="dn2")
                        nc.vector.memset(dn2, 0.0)
                    for mc2 in range(MC2):
                        pb8 = ppool.tile([128, 2, NBS], FP8, tag="p")
                        for j in range(2):
                            mc = mc2 * 2 + j
                            sp = ps_s.tile([128, 512], FP32, tag="s")
                            for kt2 in range(2):
                                nc.tensor.matmul(
                                    sp,
                                    lhsT=k8[:, kt2, :, mc * 128:(mc + 1) * 128],
                                    rhs=q8[:, kt2],
                                    start=(kt2 == 0), stop=(kt2 == 1),
                                    perf_mode=PM,
                                )
                            nc.scalar.activation(
                                pb8[:, j], sp, AF.Exp, scale=float(SCALE),
                                bias=cbias,
                            )
                        if DEBUG_DUMP and nb == 0 and mc2 == 0:
                            t = ypool.tile([128, 1024], FP32, tag="dbgp",
                                           name="dbgp")
                            nc.vector.tensor_copy(t[:, 0:512], pb8[:, 0])
                            nc.vector.tensor_copy(t[:, 512:1024], pb8[:, 1])
                            nc.sync.dma_start(out=dbg["dbg_p"][:], in_=t)
                        if mc2 % 8 != 7 and not last:
                            nc.vector.tensor_add(dn2, dn2, pb8)
                        else:
                            # last block: all-PE denominator so the exposed
                            # tail skips the DVE fold chain entirely
                            nc.tensor.matmul(
                                dn_ps, lhsT=ones8, rhs=pb8,
                                start=(mc2 == (0 if last else 7)),
                                stop=(last and mc2 == MC2 - 1),
                                perf_mode=PM, skip_group_check=True,
                            )
                        for et in range(CT):
                            nc.tensor.matmul(
                                o_ps[et],
                                lhsT=v2[:, mc2, :, et * 128:(et + 1) * 128],
                                rhs=pb8,
                                start=(mc2 == 0), stop=(mc2 == MC2 - 1),
                                perf_mode=PM,
                            )

                    # fold DVE's dn2 planes, round to f32r, and combine into
                    # dn_ps (partition-reduce + broadcast) with the group's
                    # stop.  Then reciprocal as exp(-ln(dn)) on ACT: DVE's
                    # reciprocal is ~6.6 cycles/elem; divide/custom-DVE ISA
                    # don't pass this walrus.  dn ~ 300 so ln/exp roundtrip
                    # is accurate to ~1e-6 rel.
                    if not last:
                        dnf = dnpool.tile([128, NBS], FP32, tag="dnf",
                                          name="dnf")
                        nc.vector.tensor_add(dnf, dn2[:, 0], dn2[:, 1])
                        dnr = dnpool.tile([128, NBS], F32R, tag="dnr",
                                          name="dnr")
                        nc.vector.tensor_copy(dnr, dnf)
                        nc.tensor.matmul(
                            dn_ps, lhsT=ones_r, rhs=dnr,
                            start=False, stop=True, skip_group_check=True,
                        )
                    lnd = dnpool.tile([128, NBS], FP32, tag="lnd", name="lnd")
                    nc.scalar.activation(lnd, dn_ps, AF.Ln)
                    rb = dnpool.tile([128, NBS], FP32, tag="rb", name="rb")
                    nc.scalar.activation(rb, lnd, AF.Exp, scale=-1.0)

                    q8_cur = emit_qproj(nb + 1) if nb + 1 < NB else None
                    if DEBUG_DUMP and nb == 0:
                        t = ypool.tile([128, 512], FP32, tag="dbgdn",
                                       name="dbgdn")
                        nc.vector.tensor_copy(t, dn_ps)
                        nc.sync.dma_start(out=dbg["dbg_dn"][:], in_=t)
                        nc.sync.dma_start(out=dbg["dbg_rb"][:], in_=rb)
                        t = ypool.tile([128, 512], FP32, tag="dbgq",
                                       name="dbgq")
                        nc.vector.tensor_copy(t, q8[:, 0, 0, :])
                        nc.sync.dma_start(out=dbg["dbg_q"][:], in_=t)

                    halves = ([slice(0, 256), slice(256, 512)]
                              if nb == NB - 1 else [slice(0, NBS)])
                    for et in range(CT):
                        yt = ypool.tile([128, NBS], FP32, tag="y")
                        for hv in halves:
                            nv = slice(nb * NBS + hv.start, nb * NBS + hv.stop)
                            # y = O*rb + boeff2 + x
                            nc.vector.tensor_tensor(
                                yt[:, hv], o_ps[et][:, hv], rb[:, hv],
                                op=ALU.mult
                            )
                            nc.vector.scalar_tensor_tensor(
                                yt[:, hv], yt[:, hv], bo2_sb[:, et:et + 1],
                                x_sb[:, et, nv], op0=ALU.add, op1=ALU.add,
                            )
                            qs_eng[(nb * 4 + et) % 3].dma_start(
                                out=y_r[et][:, nv], in_=yt[:, hv]
                            )

    if os.environ.get("ATTN_NO_SPLIT", "0") != "1":
        _split_multi_waits(nc)
    return nc


_NC_CACHE = {}


def _get_nc():
    key = DEBUG_DUMP
    if key not in _NC_CACHE:
        _NC_CACHE[key] = _build_kernel()
    return _NC_CACHE[key]


def _to_fp8_pairs(w):
    """[512, 512] fp32 (contraction dim first) -> [128, 2, 2, 512] TRN-e4m3
    pair layout: out[p, ct2, i, d] = w[ct2*256 + i*128 + p, d]."""
    w = np.asarray(w, np.float32).reshape(2, 2, 128, C).transpose(2, 0, 1, 3)
    w = np.clip(w, -240.0, 240.0)
    return np.ascontiguousarray(w).astype(ml_dtypes.float8_e4m3)


def _pc(v):
    """[512] -> [128, 4] with channel = t*128 + p."""
    return np.ascontiguousarray(
        np.asarray(v, np.float32).reshape(CT, 128).T
    )


def _make_in_maps(x, gn_w, gn_b, wq, bq, wk, bk, wv, bv, wo, bo):
    x = np.asarray(x, np.float32).reshape(B, C, N)
    wqT = np.asarray(wq, np.float32).T            # [c, d]
    wkT = np.asarray(wk, np.float32).T            # [c, d]
    wvT = np.asarray(wv, np.float32).T            # [c, d]
    woT = np.asarray(wo, np.float32).T            # [d, e]
    wvoT = wvT @ woT                              # [c, e]: V'' = h @ wvoT
    boeff2 = np.asarray(bo, np.float32) + np.asarray(bv, np.float32) @ woT

    shared = {
        "wq8": _to_fp8_pairs(wqT),
        "wk8": _to_fp8_pairs(wkT),
        "wvo8": _to_fp8_pairs(wvoT),
        "gnw": _pc(gn_w),
        "gnb": _pc(gn_b),
        "bqp": _pc(bq),
        "bo2": _pc(boeff2),
    }
    ind128 = np.zeros((128, 2), np.float32)
    ind128[:64, 0] = 1.0 / 64.0
    ind128[64:, 1] = 1.0 / 64.0
    indT2 = np.zeros((128, 128), np.float32)
    indT2[0, :64] = 1.0
    indT2[1, 64:] = 1.0
    shared["ind128"] = ind128
    shared["indT2"] = indT2
    x16 = x.astype(ml_dtypes.bfloat16)
    return [
        {"x": np.ascontiguousarray(x16[b]), **shared} for b in range(B)
    ]


def run(inputs, trace=False, tmpdir=None):
    nc = _get_nc()
    in_maps = _make_in_maps(**inputs)
    res = run_bass_kernel_spmd(
        nc, in_maps, core_ids=list(range(B)), trace=trace, tmpdir=tmpdir
    )
    out = np.stack([res.results[b]["y"] for b in range(B)])
    return out.reshape(B, C, 64, 64).astype(np.float32), res


def kernel(**inputs):
    out, _ = run(inputs)
    return out


# revision 45
# speedup vs baseline: 1.2434x; 1.2434x over previous
"""Trainium2 Bass kernel for an AttentionBlock (GroupNorm + single-head
self-attention + residual) over x[8, 512, 64, 64].

Sharding: data-parallel over batch - one batch element per NeuronCore (8
cores), no collectives.  Per-core layout is channel-major [C=512, N=4096].

All heavy matmuls run as fp8(TRN e4m3) DoubleRow (perf-mode) matmuls:
effective K=256 per instruction at 2 moving rows/cycle - measured ~2.5x
the f32r MAC rate on this silicon (~137ns compute + ~80ns weight load per
512-row instruction; the 512-row output cap is an ISA limit, so ~215ns
per matmul is the per-instruction floor).

Structure (vs the f32r baseline this replaced):
  - GroupNorm is folded into the *activations*: one DVE pass makes
    xn = a*x + beff in fp8 pair layout [p, ct2, i, m].  All weights are
    then constants, pre-quantized to fp8 pair layout on the host.
  - The output projection is folded into V on the host (wvo = wvT @ woT),
    so V'' = xn @ wvo directly; no second projection on device.  bo/bv
    fold into a final additive constant (softmax weights sum to 1).
  - The K-side bias bk cancels in the softmax; only bq survives (on Q').
  - x is loaded once as bf16 (halves the DMA serial head; stats/xn/
    residual all read the resident copy).  The xn pass is split ACT/DVE
    (2 channel tiles each) so DVE doesn't pace phase 3.
  - exp evicts fp8 with a constant offset exp(s*scale - 2.5): cancels
    exactly in the softmax, keeps max P ~ e^3.6 << 240 (TRN e4m3
    overflows to Inf at 256).
  - Softmax denominator: 12 of 16 key pair-chunks accumulate on DVE
    (fp8 P pairs, one 1024-wide add each); 4 accumulate on the PE via a
    broadcast ones-matmul into a held PSUM bank.  A single f32r
    ones-matmul folds the DVE partials in (partition-reduce + broadcast),
    and the reciprocal is exp(-ln(dn)) on ACT (DVE reciprocal is ~6.6
    cycles/elem; DVE divide / custom-DVE ISA don't pass this walrus).
    The last block runs its denominator fully on the PE so its exposed
    tail skips the DVE fold chain.
  - PSUM: 4 scores banks (cycling, shared with Q-projection and the
    f32r fold) + 1 held denominator bank + 4 PV accumulators taken
    across the whole query block.
  - y = O*rb + boeff2 + x on DVE straight from PSUM, DMA'd per channel
    tile across the three DMA queues (SP/ACT/gpsimd).

Engine budget per core (of ~347us): PE ~287us (the bottleneck; >95%
busy steady-state at the chip's ~86% power-throttle duty), DVE ~230us,
ACT ~202us.  Serial head ~35us (x DMA on 3 queues + GroupNorm stats),
drain tail ~18us.

Measured (8 cores, NTFF): ~346us HW exec, rel err 5.1e-3 vs fp32
reference (gate 2e-2; error budget is dominated by fp8 rounding of the
attention path, attenuated ~40x by the residual).  f32r baseline was
776-918us.
"""

import os

import ml_dtypes
import numpy as np

import concourse.bass as bass
import concourse.mybir as mybir
import concourse.tile as tile

from concourse.bass_utils import run_bass_kernel_spmd
from concourse.vector_clock import ScopedClock

AF = mybir.ActivationFunctionType
ALU = mybir.AluOpType
FP32 = mybir.dt.float32
F32R = mybir.dt.float32r
FP8 = mybir.dt.float8e4
BF16 = mybir.dt.bfloat16
PM = mybir.MatmulPerfMode.DoubleRow

B = 8
C = 512
N = 4096          # H*W
G = 8             # groups
EPS = 1e-5
CT = C // 128     # 4 channel tiles
NBS = 512         # query-block size
NB = N // NBS     # 8 query blocks
MC2 = N // 256    # 16 key pair-chunks
SCALE = 1.0 / np.sqrt(np.float32(C))
EXP_OFF = 2.5     # exp(s*SCALE - EXP_OFF); cancels in softmax exactly

DEBUG_DUMP = os.environ.get("ATTN_DEBUG_DUMP", "0") == "1"


class _TileContext(tile.TileContext):
    """This container's walrus rejects >1 sync wait on a CTRL instruction
    ("Too many sync wait commands"); split the tail drain's waits across
    multiple drain instructions.  It also rejects long semaphore-range-clear
    ISA instructions ("ISA wrong length"); clear in chunks of <=3."""

    def _drain_and_barrier(self, tick_clock, wait_clock):
        drain_inst = self.nc.sync.drain()
        wait_clock.add_sem_waits(
            drain_inst.ins, ScopedClock({None: tick_clock.global_clock})
        )
        si = drain_inst.ins.sync_info
        if si is not None and si.on_wait and len(si.on_wait) > 1:
            waits = list(si.on_wait)
            drain_inst.ins.sync_info = mybir.SyncInfo(
                on_wait=[waits[0]], on_update=list(si.on_update)
            )
            for w in waits[1:]:
                d = self.nc.sync.drain()
                d.ins.sync_info = mybir.SyncInfo(on_wait=[w], on_update=[])

        self.nc.all_engine_barrier()
        assert self.sems is not None
        popped = self.nc._tile_sem_poison_stack.pop()
        assert popped is self._sem_poison
        sems = list(self.sems.allocated().values())
        for i in range(0, len(sems), 3):
            self.nc.clear_and_free_semaphores(sems[i:i + 3])
        self.nc.all_engine_barrier()


def _split_multi_waits(nc, limit=1):
    """This container's walrus accepts at most one sync wait per instruction.
    Hoist extra waits onto same-engine EventSemaphore instructions inserted
    just before - equivalent ordering (engines execute in program order)."""
    nid = 0
    for f in nc.m.functions:
        for bb in f.blocks:
            out = []
            changed = False
            for inst in bb.instructions:
                si = inst.sync_info
                if si is not None and si.on_wait and len(si.on_wait) > limit:
                    waits = list(si.on_wait)
                    for w in waits[:-limit]:
                        ev = mybir.InstEventSemaphore(
                            name=f"I-wsplit-{nid}",
                            engine=inst.engine,
                            sync_info=mybir.SyncInfo(on_wait=[w], on_update=[]),
                        )
                        nid += 1
                        out.append(ev)
                    inst.sync_info = mybir.SyncInfo(
                        on_wait=waits[-limit:], on_update=list(si.on_update)
                    )
                    changed = True
                out.append(inst)
            if changed:
                bb.instructions = out


def _build_kernel():
    nc = bass.Bass()

    x = nc.declare_dram_parameter("x", [C, N], BF16, isOutput=False)
    # fp8 pair-layout weights: [p, ct2, i, d], channel c = ct2*256 + i*128 + p
    wq8 = nc.declare_dram_parameter("wq8", [128, 2, 2, C], FP8, isOutput=False)
    wk8 = nc.declare_dram_parameter("wk8", [128, 2, 2, C], FP8, isOutput=False)
    wvo8 = nc.declare_dram_parameter("wvo8", [128, 2, 2, C], FP8, isOutput=False)
    gnw = nc.declare_dram_parameter("gnw", [128, CT], FP32, isOutput=False)
    gnb = nc.declare_dram_parameter("gnb", [128, CT], FP32, isOutput=False)
    bqp = nc.declare_dram_parameter("bqp", [128, CT], FP32, isOutput=False)
    bo2 = nc.declare_dram_parameter("bo2", [128, CT], FP32, isOutput=False)
    # group-indicator constants for the cross-partition GroupNorm reductions
    ind128 = nc.declare_dram_parameter("ind128", [128, 2], FP32, isOutput=False)
    indT2 = nc.declare_dram_parameter("indT2", [128, 128], FP32, isOutput=False)
    y = nc.declare_dram_parameter("y", [C, N], FP32, isOutput=True)
    dbg = {}
    if DEBUG_DUMP:
        for nm, shp in [
            ("dbg_ab", [128, 2 * CT]),     # a_pc | beff
            ("dbg_xn", [128, 512]),        # xn8[:, 0, 0, :512]
            ("dbg_k", [128, 512]),         # k8[:, 0, 0, :512]
            ("dbg_q", [128, 512]),         # q8 block0 [:, 0, 0, :]
            ("dbg_v", [128, 512]),         # v2[:, 0, 0, :]
            ("dbg_p", [128, 1024]),        # pb8 block0 mc2=0
            ("dbg_dn", [128, 512]),        # dn2 folded, block 0
            ("dbg_rb", [128, 512]),        # reciprocal broadcast, block 0
        ]:
            dbg[nm] = nc.declare_dram_parameter(nm, shp, FP32, isOutput=True)

    x_r = x[:].rearrange("(t p) m -> t p m", p=128)   # [4, 128, 4096]
    y_r = y[:].rearrange("(t p) m -> t p m", p=128)

    with _TileContext(nc) as tc:
        with (
            tc.tile_pool(name="small", bufs=1) as small,
            tc.tile_pool(name="big", bufs=1) as big,
        ):
            # ---------------- persistent tiles ----------------
            x_sb = big.tile([128, CT, N], BF16, tag="x")       # 32KB/part
            xn8 = big.tile([128, 2, 2, N], FP8, tag="xn")      # 16KB/part
            k8 = big.tile([128, 2, 2, N], FP8, tag="k8")       # 16KB/part
            v2 = big.tile([128, MC2, 2, C], FP8, tag="v2")     # 16KB/part
            wq_sb = small.tile([128, 2, 2, C], FP8, tag="wq8")
            wk_sb = small.tile([128, 2, 2, C], FP8, tag="wk8")
            wvo_sb = small.tile([128, 2, 2, C], FP8, tag="wvo8")

            # x loads: 16 chunks of [128, 1024] spread over 4 DMA queues so
            # the stats head is DMA-limited as briefly as possible.
            qs_eng = [nc.sync, nc.scalar, nc.gpsimd]
            for ct in range(CT):
                for h in range(2):
                    hs = slice(h * 2048, (h + 1) * 2048)
                    qs_eng[(ct * 2 + h) % 3].dma_start(
                        out=x_sb[:, ct, hs], in_=x_r[ct][:, hs]
                    )

            nc.sync.dma_start(out=wq_sb, in_=wq8[:])
            nc.sync.dma_start(out=wk_sb, in_=wk8[:])
            nc.sync.dma_start(out=wvo_sb, in_=wvo8[:])

            ind128_sb = small.tile([128, 2], FP32, tag="ind128")
            indT2_sb = small.tile([128, 128], FP32, tag="indT2")
            nc.gpsimd.dma_start(out=ind128_sb, in_=ind128[:])
            nc.gpsimd.dma_start(out=indT2_sb, in_=indT2[:])

            def load_pc(name, dram):
                t = small.tile([128, CT], FP32, tag=name)
                nc.gpsimd.dma_start(out=t, in_=dram[:])
                return t

            gnw_sb = load_pc("gnw", gnw)
            gnb_sb = load_pc("gnb", gnb)
            bq_sb = load_pc("bqp", bqp)
            bo2_sb = load_pc("bo2", bo2)

            eps_sb = small.tile([128, 1], FP32, tag="eps")
            nc.vector.memset(eps_sb, EPS)
            cbias = small.tile([128, 1], FP32, tag="cbias")
            nc.vector.memset(cbias, -float(EXP_OFF))
            ones_f = small.tile([128, 128], FP32, tag="ones_f")
            nc.vector.memset(ones_f, 1.0)
            ones_r = small.tile([128, 128], F32R, tag="ones_r")
            nc.vector.tensor_copy(ones_r, ones_f)
            # fp8 pair-ones for the PE-side denominator accumulation; M=128
            # so the denominator lands broadcast across all 128 partitions
            # (output rows are what matmuls cost, M is free).
            ones8 = small.tile([128, 2, 128], FP8, tag="ones8")
            nc.vector.tensor_copy(ones8[:, 0, :], ones_f)
            nc.vector.tensor_copy(ones8[:, 1, :], ones_f)

            pcs = small.tile([128, 8], FP32, tag="pcs")        # (s,t): s*4+t
            stats128 = small.tile([128, 8], FP32, tag="st128")  # (j,t): j*4+t
            a_pc = small.tile([128, CT], FP32, tag="a_pc")
            beff = small.tile([128, CT], FP32, tag="beff")

            # ============ phase 1: GroupNorm statistics =============
            with (
                tc.tile_pool(name="ps3", bufs=4, space="PSUM") as ps_mm,
                tc.tile_pool(name="sttmp", bufs=4) as sttmp,
            ):
                for ct in range(CT):
                    st = sttmp.tile([128, 8, 6], FP32, tag="st")
                    for j in range(8):
                        nc.vector.bn_stats(
                            out=st[:, j], in_=x_sb[:, ct, j * 512:(j + 1) * 512]
                        )
                    mv = sttmp.tile([128, 2], FP32, tag="mv")
                    nc.vector.bn_aggr(out=mv, in_=st)
                    # pcs[:, ct]=mean ; pcs[:, 4+ct]=E[x^2]=var+mean^2
                    nc.vector.tensor_copy(pcs[:, ct:ct + 1], mv[:, 0:1])
                    m2 = sttmp.tile([128, 1], FP32, tag="m2")
                    nc.vector.tensor_mul(m2, mv[:, 0:1], mv[:, 0:1])
                    nc.vector.tensor_add(
                        pcs[:, 4 + ct:5 + ct], mv[:, 1:2], m2
                    )

                # group sums over the 64 member channels' stats
                gs_ps = ps_mm.tile([128, 512], FP32, tag="mm")
                nc.tensor.matmul(
                    gs_ps[:2, :8], lhsT=ind128_sb, rhs=pcs, start=True, stop=True
                )
                # ind128 carries the 1/64 group-mean scale (host-folded)
                gs_sb = small.tile([128, 8], FP32, tag="gs")
                nc.vector.tensor_copy(gs_sb[:2], gs_ps[:2, :8])
                nc.vector.memset(stats128, 0.0)
                nc.vector.tensor_copy(stats128[:2, 0:4], gs_sb[:2, 0:4])
                vtmp = small.tile([128, 4], FP32, tag="vtmp")
                nc.vector.tensor_mul(vtmp[:2], gs_sb[:2, 0:4], gs_sb[:2, 0:4])
                nc.vector.tensor_sub(
                    stats128[:2, 4:8], gs_sb[:2, 4:8], vtmp[:2]
                )
                nc.scalar.activation(
                    stats128[:2, 4:8], stats128[:2, 4:8], AF.Sqrt,
                    bias=eps_sb[:2],
                )
                nc.vector.reciprocal(stats128[:2, 4:8], stats128[:2, 4:8])

                # broadcast group stats back to channels
                bc_ps = ps_mm.tile([128, 512], FP32, tag="mm")
                nc.tensor.matmul(
                    bc_ps[:, :8], lhsT=indT2_sb, rhs=stats128,
                    start=True, stop=True,
                )
                # a = rstd * gn_w ; beff = gn_b - mean * a  (PSUM-direct)
                nc.vector.tensor_mul(a_pc, bc_ps[:, 4:8], gnw_sb)
                nc.vector.tensor_mul(beff, bc_ps[:, 0:4], a_pc)
                nc.vector.tensor_sub(beff, gnb_sb, beff)

            if DEBUG_DUMP:
                ab = small.tile([128, 2 * CT], FP32, tag="dbg_ab")
                nc.vector.tensor_copy(ab[:, 0:CT], a_pc)
                nc.vector.tensor_copy(ab[:, CT:2 * CT], beff)
                nc.sync.dma_start(out=dbg["dbg_ab"][:], in_=ab)

            # ===== phases 2+3 interleaved per 1024-token slab ==========
            ps3b_ctx = tc.tile_pool(name="ps3b", bufs=4, space="PSUM")
            ps_mm = ps3b_ctx.__enter__()
            # xn8[p, ct2, i, m] = a[c]x[c, m] + beff[c],  c = ct2*256+i*128+p
            # then K' / V'' for that slab, so the PE starts ~5us after the
            # GroupNorm statistics instead of after the whole xn pass.
            # K'[d, m] (d = dt*128+p, pair layout over dt) - no bias (bk
            # cancels in softmax).  V''[m, e] = xn^T wvo (pair over tokens).
            for h in range(4):
                subs = ([slice(0, 512), slice(512, 1024)] if h == 0
                        else [slice(0, 1024)])
                for sub in subs:
                    hs = slice(h * 1024 + sub.start, h * 1024 + sub.stop)
                    for ct in range(CT):
                        ct2, i = divmod(ct, 2)
                        if ct < 2:
                            # ACT carries half the xn pass (Identity with
                            # per-partition scale/bias APs): DVE otherwise
                            # paces phase 3 with xn + V'' evictions
                            nc.scalar.activation(
                                xn8[:, ct2, i, hs], x_sb[:, ct, hs],
                                AF.Identity, bias=beff[:, ct:ct + 1],
                                scale=a_pc[:, ct:ct + 1],
                            )
                        else:
                            nc.vector.tensor_scalar(
                                xn8[:, ct2, i, hs], x_sb[:, ct, hs],
                                a_pc[:, ct:ct + 1], beff[:, ct:ct + 1],
                                op0=ALU.mult, op1=ALU.add,
                            )
                for m2 in (2 * h, 2 * h + 1):
                    sl = slice(m2 * 512, (m2 + 1) * 512)
                    for dt in range(CT):
                        kp = ps_mm.tile([128, 512], FP32, tag="mm")
                        for ct2 in range(2):
                            nc.tensor.matmul(
                                kp,
                                lhsT=wk_sb[:, ct2, :, dt * 128:(dt + 1) * 128],
                                rhs=xn8[:, ct2, :, sl],
                                start=(ct2 == 0), stop=(ct2 == 1),
                                perf_mode=PM,
                            )
                        nc.scalar.copy(k8[:, dt // 2, dt % 2, sl], kp)
                    for mt in range(4):
                        mg = m2 * 4 + mt
                        msl = slice(mg * 128, (mg + 1) * 128)
                        vp = ps_mm.tile([128, 512], FP32, tag="mm")
                        for ct2 in range(2):
                            nc.tensor.matmul(
                                vp,
                                lhsT=xn8[:, ct2, :, msl],
                                rhs=wvo_sb[:, ct2],
                                start=(ct2 == 0), stop=(ct2 == 1),
                                perf_mode=PM,
                            )
                        nc.vector.tensor_copy(v2[:, mg // 2, mg % 2, :], vp)

            ps3b_ctx.__exit__(None, None, None)

            if DEBUG_DUMP:
                t = small.tile([128, 512], FP32, tag="dbg_xn")
                nc.vector.tensor_copy(t, xn8[:, 0, 0, 0:512])
                nc.sync.dma_start(out=dbg["dbg_xn"][:], in_=t)

            if DEBUG_DUMP:
                t = small.tile([128, 512], FP32, tag="dbg_k")
                nc.vector.tensor_copy(t, k8[:, 0, 0, 0:512])
                nc.sync.dma_start(out=dbg["dbg_k"][:], in_=t)
                t = small.tile([128, 512], FP32, tag="dbg_v")
                nc.vector.tensor_copy(t, v2[:, 0, 0, :])
                nc.sync.dma_start(out=dbg["dbg_v"][:], in_=t)

            # ============ phase 4: attention per query block ===========
            with (
                tc.tile_pool(name="qp", bufs=3) as qpool,
                tc.tile_pool(name="pp", bufs=8) as ppool,
                tc.tile_pool(name="dn", bufs=3) as dnpool,
                tc.tile_pool(name="yp", bufs=3) as ypool,
                tc.tile_pool(name="ps_S", bufs=3, space="PSUM") as ps_s,
                tc.tile_pool(name="ps_dn", bufs=1, space="PSUM") as ps_dn,
                tc.tile_pool(name="ps_O", bufs=4, space="PSUM") as ps_o,
            ):
                def emit_qproj(nb):
                    """Q' projection for block nb (fp8 pair layout), with
                    +bq folded in on the DVE eviction."""
                    nsl_q = slice(nb * NBS, (nb + 1) * NBS)
                    q8 = qpool.tile([128, 2, 2, NBS], FP8, tag="q",
                                    name=f"q{nb}")
                    for dt in range(CT):
                        qp_ps = ps_s.tile([128, 512], FP32, tag="s",
                                          name=f"qps{nb}_{dt}")
                        for ct2 in range(2):
                            nc.tensor.matmul(
                                qp_ps,
                                lhsT=wq_sb[:, ct2, :, dt * 128:(dt + 1) * 128],
                                rhs=xn8[:, ct2, :, nsl_q],
                                start=(ct2 == 0), stop=(ct2 == 1),
                                perf_mode=PM,
                            )
                        nc.vector.tensor_scalar_add(
                            q8[:, dt // 2, dt % 2, :], qp_ps,
                            bq_sb[:, dt:dt + 1],
                        )
                    return q8

                q8_cur = emit_qproj(0)

                for nb in range(NB):
                    nsl = slice(nb * NBS, (nb + 1) * NBS)
                    q8 = q8_cur

                    o_ps = [
                        ps_o.tile([128, 512], FP32, tag="o", name=f"o_ps{dt}")
                        for dt in range(CT)
                    ]
                    # denominator: even key pair-chunks accumulate on DVE
                    # (dn2), odd ones on the PE via a broadcast ones-matmul
                    # (dn_ps, one held ps_s buffer); combined at the tail by
                    # a f32r ones-matmul that also reduces dn2's partitions.
                    dn_ps = ps_dn.tile([128, 512], FP32, tag="dn",
                                       name=f"dn_ps{nb}")
                    last = nb == NB - 1
                    dn2 = None
                    if not last:
                        dn2 = dnpool.tile([128, 2, NBS], FP32, tag="dn2",
                                          name="dn2")
                        nc.vector.memset(dn2, 0.0)
                    for mc2 in range(MC2):
                        pb8 = ppool.tile([128, 2, NBS], FP8, tag="p")
                        for j in range(2):
                            mc = mc2 * 2 + j
                            sp = ps_s.tile([128, 512], FP32, tag="s")
                            for kt2 in range(2):
                                nc.tensor.matmul(
                                    sp,
                                    lhsT=k8[:, kt2, :, mc * 128:(mc + 1) * 128],
                                    rhs=q8[:, kt2],
                                    start=(kt2 == 0), stop=(kt2 == 1),
                                    perf_mode=PM,
                                )
                            nc.scalar.activation(
                                pb8[:, j], sp, AF.Exp, scale=float(SCALE),
                                bias=cbias,
                            )
                        if DEBUG_DUMP and nb == 0 and mc2 == 0:
                            t = ypool.tile([128, 1024], FP32, tag="dbgp",
                                           name="dbgp")
                            nc.vector.tensor_copy(t[:, 0:512], pb8[:, 0])
                            nc.vector.tensor_copy(t[:, 512:1024], pb8[:, 1])
                            nc.sync.dma_start(out=dbg["dbg_p"][:], in_=t)
                        if mc2 % 4 != 3 and not last:
                            nc.vector.tensor_add(dn2, dn2, pb8)
                        else:
                            # last block: all-PE denominator so the exposed
                            # tail skips the DVE fold chain entirely
                            nc.tensor.matmul(
                                dn_ps, lhsT=ones8, rhs=pb8,
                                start=(mc2 == (0 if last else 3)),
                                stop=(last and mc2 == MC2 - 1),
                                perf_mode=PM, skip_group_check=True,
                            )
                        for et in range(CT):
                            nc.tensor.matmul(
                                o_ps[et],
                                lhsT=v2[:, mc2, :, et * 128:(et + 1) * 128],
                                rhs=pb8,
                                start=(mc2 == 0), stop=(mc2 == MC2 - 1),
                                perf_mode=PM,
                            )

                    # fold DVE's dn2 planes, round to f32r, and combine into
                    # dn_ps (partition-reduce + broadcast) with the group's
                    # stop.  Then reciprocal as exp(-ln(dn)) on ACT: DVE's
                    # reciprocal is ~6.6 cycles/elem; divide/custom-DVE ISA
                    # don't pass this walrus.  dn ~ 300 so ln/exp roundtrip
                    # is accurate to ~1e-6 rel.
                    if not last:
                        dnf = dnpool.tile([128, NBS], FP32, tag="dnf",
                                          name="dnf")
                        nc.vector.tensor_add(dnf, dn2[:, 0], dn2[:, 1])
                        dnr = dnpool.tile([128, NBS], F32R, tag="dnr",
                                          name="dnr")
                        nc.vector.tensor_copy(dnr, dnf)
                        nc.tensor.matmul(
                            dn_ps, lhsT=ones_r, rhs=dnr,
                            start=False, stop=True, skip_group_check=True,
                        )
                    lnd = dnpool.tile([128, NBS], FP32, tag="lnd", name="lnd")
                    nc.scalar.activation(lnd, dn_ps, AF.Ln)
                    rb = dnpool.tile([128, NBS], FP32, tag="rb", name="rb")
                    nc.scalar.activation(rb, lnd, AF.Exp, scale=-1.0)

                    q8_cur = emit_qproj(nb + 1) if nb + 1 < NB else None
                    if DEBUG_DUMP and nb == 0:
                        t = ypool.tile([128, 512], FP32, tag="dbgdn",
                                       name="dbgdn")
                        nc.vector.tensor_copy(t, dn_ps)
                        nc.sync.dma_start(out=dbg["dbg_dn"][:], in_=t)
                        nc.sync.dma_start(out=dbg["dbg_rb"][:], in_=rb)
                        t = ypool.tile([128, 512], FP32, tag="dbgq",
                                       name="dbgq")
                        nc.vector.tensor_copy(t, q8[:, 0, 0, :])
                        nc.sync.dma_start(out=dbg["dbg_q"][:], in_=t)

                    halves = ([slice(0, 256), slice(256, 512)]
                              if nb == NB - 1 else [slice(0, NBS)])
                    for et in range(CT):
                        yt = ypool.tile([128, NBS], FP32, tag="y")
                        for hv in halves:
                            nv = slice(nb * NBS + hv.start, nb * NBS + hv.stop)
                            # y = O*rb + boeff2 + x
                            nc.vector.tensor_tensor(
                                yt[:, hv], o_ps[et][:, hv], rb[:, hv],
                                op=ALU.mult
                            )
                            nc.vector.scalar_tensor_tensor(
                                yt[:, hv], yt[:, hv], bo2_sb[:, et:et + 1],
                                x_sb[:, et, nv], op0=ALU.add, op1=ALU.add,
                            )
                            qs_eng[(nb * 4 + et) % 3].dma_start(
                                out=y_r[et][:, nv], in_=yt[:, hv]
                            )

    if os.environ.get("ATTN_NO_SPLIT", "0") != "1":
        _split_multi_waits(nc)
    return nc


_NC_CACHE = {}


def _get_nc():
    key = DEBUG_DUMP
    if key not in _NC_CACHE:
        _NC_CACHE[key] = _build_kernel()
    return _NC_CACHE[key]


def _to_fp8_pairs(w):
    """[512, 512] fp32 (contraction dim first) -> [128, 2, 2, 512] TRN-e4m3
    pair layout: out[p, ct2, i, d] = w[ct2*256 + i*128 + p, d]."""
    w = np.asarray(w, np.float32).reshape(2, 2, 128, C).transpose(2, 0, 1, 3)
    w = np.clip(w, -240.0, 240.0)
    return np.ascontiguousarray(w).astype(ml_dtypes.float8_e4m3)


def _pc(v):
    """[512] -> [128, 4] with channel = t*128 + p."""
    return np.ascontiguousarray(
        np.asarray(v, np.float32).reshape(CT, 128).T
    )


def _make_in_maps(x, gn_w, gn_b, wq, bq, wk, bk, wv, bv, wo, bo):
    x = np.asarray(x, np.float32).reshape(B, C, N)
    wqT = np.asarray(wq, np.float32).T            # [c, d]
    wkT = np.asarray(wk, np.float32).T            # [c, d]
    wvT = np.asarray(wv, np.float32).T            # [c, d]
    woT = np.asarray(wo, np.float32).T            # [d, e]
    wvoT = wvT @ woT                              # [c, e]: V'' = h @ wvoT
    boeff2 = np.asarray(bo, np.float32) + np.asarray(bv, np.float32) @ woT

    shared = {
        "wq8": _to_fp8_pairs(wqT),
        "wk8": _to_fp8_pairs(wkT),
        "wvo8": _to_fp8_pairs(wvoT),
        "gnw": _pc(gn_w),
        "gnb": _pc(gn_b),
        "bqp": _pc(bq),
        "bo2": _pc(boeff2),
    }
    ind128 = np.zeros((128, 2), np.float32)
    ind128[:64, 0] = 1.0 / 64.0
    ind128[64:, 1] = 1.0 / 64.0
    indT2 = np.zeros((128, 128), np.float32)
    indT2[0, :64] = 1.0
    indT2[1, 64:] = 1.0
    shared["ind128"] = ind128
    shared["indT2"] = indT2
    x16 = x.astype(ml_dtypes.bfloat16)
    return [
        {"x": np.ascontiguousarray(x16[b]), **shared} for b in range(B)
    ]


def run(inputs, trace=False, tmpdir=None):
    nc = _get_nc()
    in_maps = _make_in_maps(**inputs)
    res = run_bass_kernel_spmd(
        nc, in_maps, core_ids=list(range(B)), trace=trace, tmpdir=tmpdir
    )
    out = np.stack([res.results[b]["y"] for b in range(B)])
    return out.reshape(B, C, 64, 64).astype(np.float32), res


def kernel(**inputs):
    out, _ = run(inputs)
    return out


# revision 46
# speedup vs baseline: 1.2485x; 1.0041x over previous
"""Trainium2 Bass kernel for an AttentionBlock (GroupNorm + single-head
self-attention + residual) over x[8, 512, 64, 64].

Sharding: data-parallel over batch - one batch element per NeuronCore (8
cores), no collectives.  Per-core layout is channel-major [C=512, N=4096].

All heavy matmuls run as fp8(TRN e4m3) DoubleRow (perf-mode) matmuls:
effective K=256 per instruction at 2 moving rows/cycle - measured ~2.5x
the f32r MAC rate on this silicon (~137ns compute + ~80ns weight load per
512-row instruction; the 512-row output cap is an ISA limit, so ~215ns
per matmul is the per-instruction floor).

Structure (vs the f32r baseline this replaced):
  - GroupNorm is folded into the *activations*: one DVE pass makes
    xn = a*x + beff in fp8 pair layout [p, ct2, i, m].  All weights are
    then constants, pre-quantized to fp8 pair layout on the host.
  - The output projection is folded into V on the host (wvo = wvT @ woT),
    so V'' = xn @ wvo directly; no second projection on device.  bo/bv
    fold into a final additive constant (softmax weights sum to 1).
  - The K-side bias bk cancels in the softmax; only bq survives (on Q').
  - x is loaded once as bf16 (halves the DMA serial head; stats/xn/
    residual all read the resident copy).  The xn pass is split ACT/DVE
    (2 channel tiles each) so DVE doesn't pace phase 3.
  - exp evicts fp8 with a constant offset exp(s*scale - 2.5): cancels
    exactly in the softmax, keeps max P ~ e^3.6 << 240 (TRN e4m3
    overflows to Inf at 256).
  - Softmax denominator: 12 of 16 key pair-chunks accumulate on DVE
    (fp8 P pairs, one 1024-wide add each); 4 accumulate on the PE via a
    broadcast ones-matmul into a held PSUM bank.  A single f32r
    ones-matmul folds the DVE partials in (partition-reduce + broadcast),
    and the reciprocal is exp(-ln(dn)) on ACT (DVE reciprocal is ~6.6
    cycles/elem; DVE divide / custom-DVE ISA don't pass this walrus).
    The last block runs its denominator fully on the PE so its exposed
    tail skips the DVE fold chain.
  - PSUM: 4 scores banks (cycling, shared with Q-projection and the
    f32r fold) + 1 held denominator bank + 4 PV accumulators taken
    across the whole query block.
  - y = O*rb + boeff2 + x on DVE straight from PSUM, DMA'd per channel
    tile across the three DMA queues (SP/ACT/gpsimd).

Engine budget per core (of ~347us): PE ~287us (the bottleneck; >95%
busy steady-state at the chip's ~86% power-throttle duty), DVE ~230us,
ACT ~202us.  Serial head ~35us (x DMA on 3 queues + GroupNorm stats),
drain tail ~18us.

Measured (8 cores, NTFF): ~346us HW exec, rel err 5.1e-3 vs fp32
reference (gate 2e-2; error budget is dominated by fp8 rounding of the
attention path, attenuated ~40x by the residual).  f32r baseline was
776-918us.
"""

import os

import ml_dtypes
import numpy as np

import concourse.bass as bass
import concourse.mybir as mybir
import concourse.tile as tile

from concourse.bass_utils import run_bass_kernel_spmd
from concourse.vector_clock import ScopedClock

AF = mybir.ActivationFunctionType
ALU = mybir.AluOpType
FP32 = mybir.dt.float32
F32R = mybir.dt.float32r
FP8 = mybir.dt.float8e4
BF16 = mybir.dt.bfloat16
PM = mybir.MatmulPerfMode.DoubleRow

B = 8
C = 512
N = 4096          # H*W
G = 8             # groups
EPS = 1e-5
CT = C // 128     # 4 channel tiles
NBS = 512         # query-block size
NB = N // NBS     # 8 query blocks
MC2 = N // 256    # 16 key pair-chunks
SCALE = 1.0 / np.sqrt(np.float32(C))
EXP_OFF = 2.5     # exp(s*SCALE - EXP_OFF); cancels in softmax exactly

DEBUG_DUMP = os.environ.get("ATTN_DEBUG_DUMP", "0") == "1"


class _TileContext(tile.TileContext):
    """This container's walrus rejects >1 sync wait on a CTRL instruction
    ("Too many sync wait commands"); split the tail drain's waits across
    multiple drain instructions.  It also rejects long semaphore-range-clear
    ISA instructions ("ISA wrong length"); clear in chunks of <=3."""

    def _drain_and_barrier(self, tick_clock, wait_clock):
        drain_inst = self.nc.sync.drain()
        wait_clock.add_sem_waits(
            drain_inst.ins, ScopedClock({None: tick_clock.global_clock})
        )
        si = drain_inst.ins.sync_info
        if si is not None and si.on_wait and len(si.on_wait) > 1:
            waits = list(si.on_wait)
            drain_inst.ins.sync_info = mybir.SyncInfo(
                on_wait=[waits[0]], on_update=list(si.on_update)
            )
            for w in waits[1:]:
                d = self.nc.sync.drain()
                d.ins.sync_info = mybir.SyncInfo(on_wait=[w], on_update=[])

        self.nc.all_engine_barrier()
        assert self.sems is not None
        popped = self.nc._tile_sem_poison_stack.pop()
        assert popped is self._sem_poison
        sems = list(self.sems.allocated().values())
        for i in range(0, len(sems), 3):
            self.nc.clear_and_free_semaphores(sems[i:i + 3])
        self.nc.all_engine_barrier()


def _split_multi_waits(nc, limit=1):
    """This container's walrus accepts at most one sync wait per instruction.
    Hoist extra waits onto same-engine EventSemaphore instructions inserted
    just before - equivalent ordering (engines execute in program order)."""
    nid = 0
    for f in nc.m.functions:
        for bb in f.blocks:
            out = []
            changed = False
            for inst in bb.instructions:
                si = inst.sync_info
                if si is not None and si.on_wait and len(si.on_wait) > limit:
                    waits = list(si.on_wait)
                    for w in waits[:-limit]:
                        ev = mybir.InstEventSemaphore(
                            name=f"I-wsplit-{nid}",
                            engine=inst.engine,
                            sync_info=mybir.SyncInfo(on_wait=[w], on_update=[]),
                        )
                        nid += 1
                        out.append(ev)
                    inst.sync_info = mybir.SyncInfo(
                        on_wait=waits[-limit:], on_update=list(si.on_update)
                    )
                    changed = True
                out.append(inst)
            if changed:
                bb.instructions = out


def _build_kernel():
    nc = bass.Bass()

    x = nc.declare_dram_parameter("x", [C, N], BF16, isOutput=False)
    # fp8 pair-layout weights: [p, ct2, i, d], channel c = ct2*256 + i*128 + p
    wq8 = nc.declare_dram_parameter("wq8", [128, 2, 2, C], FP8, isOutput=False)
    wk8 = nc.declare_dram_parameter("wk8", [128, 2, 2, C], FP8, isOutput=False)
    wvo8 = nc.declare_dram_parameter("wvo8", [128, 2, 2, C], FP8, isOutput=False)
    gnw = nc.declare_dram_parameter("gnw", [128, CT], FP32, isOutput=False)
    gnb = nc.declare_dram_parameter("gnb", [128, CT], FP32, isOutput=False)
    bqp = nc.declare_dram_parameter("bqp", [128, CT], FP32, isOutput=False)
    bo2 = nc.declare_dram_parameter("bo2", [128, CT], FP32, isOutput=False)
    # group-indicator constants for the cross-partition GroupNorm reductions
    ind128 = nc.declare_dram_parameter("ind128", [128, 2], FP32, isOutput=False)
    indT2 = nc.declare_dram_parameter("indT2", [128, 128], FP32, isOutput=False)
    y = nc.declare_dram_parameter("y", [C, N], FP32, isOutput=True)
    dbg = {}
    if DEBUG_DUMP:
        for nm, shp in [
            ("dbg_ab", [128, 2 * CT]),     # a_pc | beff
            ("dbg_xn", [128, 512]),        # xn8[:, 0, 0, :512]
            ("dbg_k", [128, 512]),         # k8[:, 0, 0, :512]
            ("dbg_q", [128, 512]),         # q8 block0 [:, 0, 0, :]
            ("dbg_v", [128, 512]),         # v2[:, 0, 0, :]
            ("dbg_p", [128, 1024]),        # pb8 block0 mc2=0
            ("dbg_dn", [128, 512]),        # dn2 folded, block 0
            ("dbg_rb", [128, 512]),        # reciprocal broadcast, block 0
        ]:
            dbg[nm] = nc.declare_dram_parameter(nm, shp, FP32, isOutput=True)

    x_r = x[:].rearrange("(t p) m -> t p m", p=128)   # [4, 128, 4096]
    y_r = y[:].rearrange("(t p) m -> t p m", p=128)

    with _TileContext(nc) as tc:
        with (
            tc.tile_pool(name="small", bufs=1) as small,
            tc.tile_pool(name="big", bufs=1) as big,
        ):
            # ---------------- persistent tiles ----------------
            x_sb = big.tile([128, CT, N], BF16, tag="x")       # 32KB/part
            xn8 = big.tile([128, 2, 2, N], FP8, tag="xn")      # 16KB/part
            k8 = big.tile([128, 2, 2, N], FP8, tag="k8")       # 16KB/part
            v2 = big.tile([128, MC2, 2, C], FP8, tag="v2")     # 16KB/part
            wq_sb = small.tile([128, 2, 2, C], FP8, tag="wq8")
            wk_sb = small.tile([128, 2, 2, C], FP8, tag="wk8")
            wvo_sb = small.tile([128, 2, 2, C], FP8, tag="wvo8")

            # x loads: 16 chunks of [128, 1024] spread over 4 DMA queues so
            # the stats head is DMA-limited as briefly as possible.
            qs_eng = [nc.sync, nc.scalar, nc.gpsimd]
            for ct in range(CT):
                for h in range(2):
                    hs = slice(h * 2048, (h + 1) * 2048)
                    qs_eng[(ct * 2 + h) % 3].dma_start(
                        out=x_sb[:, ct, hs], in_=x_r[ct][:, hs]
                    )

            # weights ride the lightest queue (gpsimd carries only 2 of the
            # 8 x chunks) so phase 3's first K matmuls don't wait on them
            nc.gpsimd.dma_start(out=wk_sb, in_=wk8[:])
            nc.gpsimd.dma_start(out=wvo_sb, in_=wvo8[:])
            nc.scalar.dma_start(out=wq_sb, in_=wq8[:])

            ind128_sb = small.tile([128, 2], FP32, tag="ind128")
            indT2_sb = small.tile([128, 128], FP32, tag="indT2")
            nc.sync.dma_start(out=ind128_sb, in_=ind128[:])
            nc.sync.dma_start(out=indT2_sb, in_=indT2[:])

            def load_pc(name, dram):
                t = small.tile([128, CT], FP32, tag=name)
                nc.sync.dma_start(out=t, in_=dram[:])
                return t

            gnw_sb = load_pc("gnw", gnw)
            gnb_sb = load_pc("gnb", gnb)
            bq_sb = load_pc("bqp", bqp)
            bo2_sb = load_pc("bo2", bo2)

            eps_sb = small.tile([128, 1], FP32, tag="eps")
            nc.vector.memset(eps_sb, EPS)
            cbias = small.tile([128, 1], FP32, tag="cbias")
            nc.vector.memset(cbias, -float(EXP_OFF))
            ones_f = small.tile([128, 128], FP32, tag="ones_f")
            nc.vector.memset(ones_f, 1.0)
            ones_r = small.tile([128, 128], F32R, tag="ones_r")
            nc.vector.tensor_copy(ones_r, ones_f)
            # fp8 pair-ones for the PE-side denominator accumulation; M=128
            # so the denominator lands broadcast across all 128 partitions
            # (output rows are what matmuls cost, M is free).
            ones8 = small.tile([128, 2, 128], FP8, tag="ones8")
            nc.vector.tensor_copy(ones8[:, 0, :], ones_f)
            nc.vector.tensor_copy(ones8[:, 1, :], ones_f)

            pcs = small.tile([128, 8], FP32, tag="pcs")        # (s,t): s*4+t
            stats128 = small.tile([128, 8], FP32, tag="st128")  # (j,t): j*4+t
            a_pc = small.tile([128, CT], FP32, tag="a_pc")
            beff = small.tile([128, CT], FP32, tag="beff")

            # ============ phase 1: GroupNorm statistics =============
            with (
                tc.tile_pool(name="ps3", bufs=4, space="PSUM") as ps_mm,
                tc.tile_pool(name="sttmp", bufs=4) as sttmp,
            ):
                for ct in range(CT):
                    st = sttmp.tile([128, 8, 6], FP32, tag="st")
                    for j in range(8):
                        nc.vector.bn_stats(
                            out=st[:, j], in_=x_sb[:, ct, j * 512:(j + 1) * 512]
                        )
                    mv = sttmp.tile([128, 2], FP32, tag="mv")
                    nc.vector.bn_aggr(out=mv, in_=st)
                    # pcs[:, ct]=mean ; pcs[:, 4+ct]=E[x^2]=var+mean^2
                    nc.vector.tensor_copy(pcs[:, ct:ct + 1], mv[:, 0:1])
                    m2 = sttmp.tile([128, 1], FP32, tag="m2")
                    nc.vector.tensor_mul(m2, mv[:, 0:1], mv[:, 0:1])
                    nc.vector.tensor_add(
                        pcs[:, 4 + ct:5 + ct], mv[:, 1:2], m2
                    )

                # group sums over the 64 member channels' stats
                gs_ps = ps_mm.tile([128, 512], FP32, tag="mm")
                nc.tensor.matmul(
                    gs_ps[:2, :8], lhsT=ind128_sb, rhs=pcs, start=True, stop=True
                )
                # ind128 carries the 1/64 group-mean scale (host-folded)
                gs_sb = small.tile([128, 8], FP32, tag="gs")
                nc.vector.tensor_copy(gs_sb[:2], gs_ps[:2, :8])
                nc.vector.memset(stats128, 0.0)
                nc.vector.tensor_copy(stats128[:2, 0:4], gs_sb[:2, 0:4])
                vtmp = small.tile([128, 4], FP32, tag="vtmp")
                nc.vector.tensor_mul(vtmp[:2], gs_sb[:2, 0:4], gs_sb[:2, 0:4])
                nc.vector.tensor_sub(
                    stats128[:2, 4:8], gs_sb[:2, 4:8], vtmp[:2]
                )
                nc.scalar.activation(
                    stats128[:2, 4:8], stats128[:2, 4:8], AF.Sqrt,
                    bias=eps_sb[:2],
                )
                nc.vector.reciprocal(stats128[:2, 4:8], stats128[:2, 4:8])

                # broadcast group stats back to channels
                bc_ps = ps_mm.tile([128, 512], FP32, tag="mm")
                nc.tensor.matmul(
                    bc_ps[:, :8], lhsT=indT2_sb, rhs=stats128,
                    start=True, stop=True,
                )
                # a = rstd * gn_w ; beff = gn_b - mean * a  (PSUM-direct)
                nc.vector.tensor_mul(a_pc, bc_ps[:, 4:8], gnw_sb)
                nc.vector.tensor_mul(beff, bc_ps[:, 0:4], a_pc)
                nc.vector.tensor_sub(beff, gnb_sb, beff)

            if DEBUG_DUMP:
                ab = small.tile([128, 2 * CT], FP32, tag="dbg_ab")
                nc.vector.tensor_copy(ab[:, 0:CT], a_pc)
                nc.vector.tensor_copy(ab[:, CT:2 * CT], beff)
                nc.sync.dma_start(out=dbg["dbg_ab"][:], in_=ab)

            # ===== phases 2+3 interleaved per 1024-token slab ==========
            ps3b_ctx = tc.tile_pool(name="ps3b", bufs=4, space="PSUM")
            ps_mm = ps3b_ctx.__enter__()
            # xn8[p, ct2, i, m] = a[c]x[c, m] + beff[c],  c = ct2*256+i*128+p
            # then K' / V'' for that slab, so the PE starts ~5us after the
            # GroupNorm statistics instead of after the whole xn pass.
            # K'[d, m] (d = dt*128+p, pair layout over dt) - no bias (bk
            # cancels in softmax).  V''[m, e] = xn^T wvo (pair over tokens).
            for h in range(4):
                subs = ([slice(0, 512), slice(512, 1024)] if h == 0
                        else [slice(0, 1024)])
                for sub in subs:
                    hs = slice(h * 1024 + sub.start, h * 1024 + sub.stop)
                    for ct in range(CT):
                        ct2, i = divmod(ct, 2)
                        if ct < 2:
                            # ACT carries half the xn pass (Identity with
                            # per-partition scale/bias APs): DVE otherwise
                            # paces phase 3 with xn + V'' evictions
                            nc.scalar.activation(
                                xn8[:, ct2, i, hs], x_sb[:, ct, hs],
                                AF.Identity, bias=beff[:, ct:ct + 1],
                                scale=a_pc[:, ct:ct + 1],
                            )
                        else:
                            nc.vector.tensor_scalar(
                                xn8[:, ct2, i, hs], x_sb[:, ct, hs],
                                a_pc[:, ct:ct + 1], beff[:, ct:ct + 1],
                                op0=ALU.mult, op1=ALU.add,
                            )
                for m2 in (2 * h, 2 * h + 1):
                    sl = slice(m2 * 512, (m2 + 1) * 512)
                    for dt in range(CT):
                        kp = ps_mm.tile([128, 512], FP32, tag="mm")
                        for ct2 in range(2):
                            nc.tensor.matmul(
                                kp,
                                lhsT=wk_sb[:, ct2, :, dt * 128:(dt + 1) * 128],
                                rhs=xn8[:, ct2, :, sl],
                                start=(ct2 == 0), stop=(ct2 == 1),
                                perf_mode=PM,
                            )
                        nc.scalar.copy(k8[:, dt // 2, dt % 2, sl], kp)
                    for mt in range(4):
                        mg = m2 * 4 + mt
                        msl = slice(mg * 128, (mg + 1) * 128)
                        vp = ps_mm.tile([128, 512], FP32, tag="mm")
                        for ct2 in range(2):
                            nc.tensor.matmul(
                                vp,
                                lhsT=xn8[:, ct2, :, msl],
                                rhs=wvo_sb[:, ct2],
                                start=(ct2 == 0), stop=(ct2 == 1),
                                perf_mode=PM,
                            )
                        nc.vector.tensor_copy(v2[:, mg // 2, mg % 2, :], vp)

            ps3b_ctx.__exit__(None, None, None)

            if DEBUG_DUMP:
                t = small.tile([128, 512], FP32, tag="dbg_xn")
                nc.vector.tensor_copy(t, xn8[:, 0, 0, 0:512])
                nc.sync.dma_start(out=dbg["dbg_xn"][:], in_=t)

            if DEBUG_DUMP:
                t = small.tile([128, 512], FP32, tag="dbg_k")
                nc.vector.tensor_copy(t, k8[:, 0, 0, 0:512])
                nc.sync.dma_start(out=dbg["dbg_k"][:], in_=t)
                t = small.tile([128, 512], FP32, tag="dbg_v")
                nc.vector.tensor_copy(t, v2[:, 0, 0, :])
                nc.sync.dma_start(out=dbg["dbg_v"][:], in_=t)

            # ============ phase 4: attention per query block ===========
            with (
                tc.tile_pool(name="qp", bufs=3) as qpool,
                tc.tile_pool(name="pp", bufs=8) as ppool,
                tc.tile_pool(name="dn", bufs=3) as dnpool,
                tc.tile_pool(name="yp", bufs=3) as ypool,
                tc.tile_pool(name="ps_S", bufs=3, space="PSUM") as ps_s,
                tc.tile_pool(name="ps_dn", bufs=1, space="PSUM") as ps_dn,
                tc.tile_pool(name="ps_O", bufs=4, space="PSUM") as ps_o,
            ):
                def emit_qproj(nb):
                    """Q' projection for block nb (fp8 pair layout), with
                    +bq folded in on the DVE eviction."""
                    nsl_q = slice(nb * NBS, (nb + 1) * NBS)
                    q8 = qpool.tile([128, 2, 2, NBS], FP8, tag="q",
                                    name=f"q{nb}")
                    for dt in range(CT):
                        qp_ps = ps_s.tile([128, 512], FP32, tag="s",
                                          name=f"qps{nb}_{dt}")
                        for ct2 in range(2):
                            nc.tensor.matmul(
                                qp_ps,
                                lhsT=wq_sb[:, ct2, :, dt * 128:(dt + 1) * 128],
                                rhs=xn8[:, ct2, :, nsl_q],
                                start=(ct2 == 0), stop=(ct2 == 1),
                                perf_mode=PM,
                            )
                        nc.vector.tensor_scalar_add(
                            q8[:, dt // 2, dt % 2, :], qp_ps,
                            bq_sb[:, dt:dt + 1],
                        )
                    return q8

                q8_cur = emit_qproj(0)

                for nb in range(NB):
                    nsl = slice(nb * NBS, (nb + 1) * NBS)
                    q8 = q8_cur

                    o_ps = [
                        ps_o.tile([128, 512], FP32, tag="o", name=f"o_ps{dt}")
                        for dt in range(CT)
                    ]
                    # denominator: even key pair-chunks accumulate on DVE
                    # (dn2), odd ones on the PE via a broadcast ones-matmul
                    # (dn_ps, one held ps_s buffer); combined at the tail by
                    # a f32r ones-matmul that also reduces dn2's partitions.
                    dn_ps = ps_dn.tile([128, 512], FP32, tag="dn",
                                       name=f"dn_ps{nb}")
                    last = nb == NB - 1
                    dn2 = None
                    if not last:
                        dn2 = dnpool.tile([128, 2, NBS], FP32, tag="dn2",
                                          name="dn2")
                        nc.vector.memset(dn2, 0.0)
                    for mc2 in range(MC2):
                        pb8 = ppool.tile([128, 2, NBS], FP8, tag="p")
                        for j in range(2):
                            mc = mc2 * 2 + j
                            sp = ps_s.tile([128, 512], FP32, tag="s")
                            for kt2 in range(2):
                                nc.tensor.matmul(
                                    sp,
                                    lhsT=k8[:, kt2, :, mc * 128:(mc + 1) * 128],
                                    rhs=q8[:, kt2],
                                    start=(kt2 == 0), stop=(kt2 == 1),
                                    perf_mode=PM,
                                )
                            nc.scalar.activation(
                                pb8[:, j], sp, AF.Exp, scale=float(SCALE),
                                bias=cbias,
                            )
                        if DEBUG_DUMP and nb == 0 and mc2 == 0:
                            t = ypool.tile([128, 1024], FP32, tag="dbgp",
                                           name="dbgp")
                            nc.vector.tensor_copy(t[:, 0:512], pb8[:, 0])
                            nc.vector.tensor_copy(t[:, 512:1024], pb8[:, 1])
                            nc.sync.dma_start(out=dbg["dbg_p"][:], in_=t)
                        if mc2 % 4 != 3 and not last:
                            nc.vector.tensor_add(dn2, dn2, pb8)
                        else:
                            # last block: all-PE denominator so the exposed
                            # tail skips the DVE fold chain entirely
                            nc.tensor.matmul(
                                dn_ps, lhsT=ones8, rhs=pb8,
                                start=(mc2 == (0 if last else 3)),
                                stop=(last and mc2 == MC2 - 1),
                                perf_mode=PM, skip_group_check=True,
                            )
                        for et in range(CT):
                            nc.tensor.matmul(
                                o_ps[et],
                                lhsT=v2[:, mc2, :, et * 128:(et + 1) * 128],
                                rhs=pb8,
                                start=(mc2 == 0), stop=(mc2 == MC2 - 1),
                                perf_mode=PM,
                            )

                    # fold DVE's dn2 planes, round to f32r, and combine into
                    # dn_ps (partition-reduce + broadcast) with the group's
                    # stop.  Then reciprocal as exp(-ln(dn)) on ACT: DVE's
                    # reciprocal is ~6.6 cycles/elem; divide/custom-DVE ISA
                    # don't pass this walrus.  dn ~ 300 so ln/exp roundtrip
                    # is accurate to ~1e-6 rel.
                    if not last:
                        dnf = dnpool.tile([128, NBS], FP32, tag="dnf",
                                          name="dnf")
                        nc.vector.tensor_add(dnf, dn2[:, 0], dn2[:, 1])
                        dnr = dnpool.tile([128, NBS], F32R, tag="dnr",
                                          name="dnr")
                        nc.vector.tensor_copy(dnr, dnf)
                        nc.tensor.matmul(
                            dn_ps, lhsT=ones_r, rhs=dnr,
                            start=False, stop=True, skip_group_check=True,
                        )
                    lnd = dnpool.tile([128, NBS], FP32, tag="lnd", name="lnd")
                    nc.scalar.activation(lnd, dn_ps, AF.Ln)
                    rb = dnpool.tile([128, NBS], FP32, tag="rb", name="rb")
                    nc.scalar.activation(rb, lnd, AF.Exp, scale=-1.0)

                    q8_cur = emit_qproj(nb + 1) if nb + 1 < NB else None
                    if DEBUG_DUMP and nb == 0:
                        t = ypool.tile([128, 512], FP32, tag="dbgdn",
                                       name="dbgdn")
                        nc.vector.tensor_copy(t, dn_ps)
                        nc.sync.dma_start(out=dbg["dbg_dn"][:], in_=t)
                        nc.sync.dma_start(out=dbg["dbg_rb"][:], in_=rb)
                        t = ypool.tile([128, 512], FP32, tag="dbgq",
                                       name="dbgq")
                        nc.vector.tensor_copy(t, q8[:, 0, 0, :])
                        nc.sync.dma_start(out=dbg["dbg_q"][:], in_=t)

                    halves = ([slice(0, 256), slice(256, 512)]
                              if nb == NB - 1 else [slice(0, NBS)])
                    for et in range(CT):
                        yt = ypool.tile([128, NBS], FP32, tag="y")
                        for hv in halves:
                            nv = slice(nb * NBS + hv.start, nb * NBS + hv.stop)
                            # y = O*rb + boeff2 + x
                            nc.vector.tensor_tensor(
                                yt[:, hv], o_ps[et][:, hv], rb[:, hv],
                                op=ALU.mult
                            )
                            nc.vector.scalar_tensor_tensor(
                                yt[:, hv], yt[:, hv], bo2_sb[:, et:et + 1],
                                x_sb[:, et, nv], op0=ALU.add, op1=ALU.add,
                            )
                            qs_eng[(nb * 4 + et) % 3].dma_start(
                                out=y_r[et][:, nv], in_=yt[:, hv]
                            )

    if os.environ.get("ATTN_NO_SPLIT", "0") != "1":
        _split_multi_waits(nc)
    return nc


_NC_CACHE = {}


def _get_nc():
    key = DEBUG_DUMP
    if key not in _NC_CACHE:
        _NC_CACHE[key] = _build_kernel()
    return _NC_CACHE[key]


def _to_fp8_pairs(w):
    """[512, 512] fp32 (contraction dim first) -> [128, 2, 2, 512] TRN-e4m3
    pair layout: out[p, ct2, i, d] = w[ct2*256 + i*128 + p, d]."""
    w = np.asarray(w, np.float32).reshape(2, 2, 128, C).transpose(2, 0, 1, 3)
    w = np.clip(w, -240.0, 240.0)
    return np.ascontiguousarray(w).astype(ml_dtypes.float8_e4m3)


def _pc(v):
    """[512] -> [128, 4] with channel = t*128 + p."""
    return np.ascontiguousarray(
        np.asarray(v, np.float32).reshape(CT, 128).T
    )


def _make_in_maps(x, gn_w, gn_b, wq, bq, wk, bk, wv, bv, wo, bo):
    x = np.asarray(x, np.float32).reshape(B, C, N)
    wqT = np.asarray(wq, np.float32).T            # [c, d]
    wkT = np.asarray(wk, np.float32).T            # [c, d]
    wvT = np.asarray(wv, np.float32).T            # [c, d]
    woT = np.asarray(wo, np.float32).T            # [d, e]
    wvoT = wvT @ woT                              # [c, e]: V'' = h @ wvoT
    boeff2 = np.asarray(bo, np.float32) + np.asarray(bv, np.float32) @ woT

    shared = {
        "wq8": _to_fp8_pairs(wqT),
        "wk8": _to_fp8_pairs(wkT),
        "wvo8": _to_fp8_pairs(wvoT),
        "gnw": _pc(gn_w),
        "gnb": _pc(gn_b),
        "bqp": _pc(bq),
        "bo2": _pc(boeff2),
    }
    ind128 = np.zeros((128, 2), np.float32)
    ind128[:64, 0] = 1.0 / 64.0
    ind128[64:, 1] = 1.0 / 64.0
    indT2 = np.zeros((128, 128), np.float32)
    indT2[0, :64] = 1.0
    indT2[1, 64:] = 1.0
    shared["ind128"] = ind128
    shared["indT2"] = indT2
    x16 = x.astype(ml_dtypes.bfloat16)
    return [
        {"x": np.ascontiguousarray(x16[b]), **shared} for b in range(B)
    ]


def run(inputs, trace=False, tmpdir=None):
    nc = _get_nc()
    in_maps = _make_in_maps(**inputs)
    res = run_bass_kernel_spmd(
        nc, in_maps, core_ids=list(range(B)), trace=trace, tmpdir=tmpdir
    )
    out = np.stack([res.results[b]["y"] for b in range(B)])
    return out.reshape(B, C, 64, 64).astype(np.float32), res


def kernel(**inputs):
    out, _ = run(inputs)
    return out
